# revision 14
# baseline (speedup 1.0000x reference)
"""AttentionBlock (GroupNorm + linear attention + proj + residual) on 8 Trainium2 cores.

Reference computation (per batch element b, C=512, HW=4096):
    h   = GroupNorm32(x) * w + b
    qkv = qkv_w @ h                       (1x1 conv == channel matmul)
    q   = softmax(q, axis=spatial) * C^-0.5
    k   = softmax(k, axis=spatial)
    ctx = k @ v^T                         [C, C]
    out = proj_w @ (ctx @ q) + proj_b + x

Sharding: data-parallel over batch B=8 -> one batch element per NeuronCore.

Kernel algebra (per core):
  - softmax(q+qb) == softmax(q): per-row bias shifts cancel; only v's qkv-bias
    matters and enters as a rank-1 correction to ctx (ctx += vb[d]).
  - exp() without max-subtraction (q,k values are O(1)); softmax denominators
    (sumq, sumk) folded into row scales of small [C,C] matrices.
  - proj_w folded in early: MT = (proj_w @ ctx')^T, so the last big GEMM is
    MT @ expq and the separate proj GEMM disappears.
  - k and v are produced directly in [n, c] (transposed) layout by using the
    h-tile as the matmul's stationary operand; no explicit transposes anywhere.
  - all large matmuls run as float32r (full PE rate at N=512, ~fp32 accuracy).
"""

import os
from contextlib import ExitStack

import numpy as np

B = 8
C = 512
H = W = 64
N = H * W  # 4096 spatial positions
P = 128  # partitions
CT = C // P  # 4 channel tiles
NT = N // P  # 32 spatial tiles of 128 (for transposed k/v)
NCH = N // 512  # 8 spatial chunks of 512
GROUPS = 32
GSIZE = C // GROUPS  # 16 channels per group
EPS = 1e-5

_CACHE = {}


def _build_program():
    import concourse.bass as bass
    import concourse.tile as tile
    from concourse import bacc, mybir
    from concourse.bass import ts

    f32 = mybir.dt.float32
    f32r = mybir.dt.float32r
    AF = mybir.ActivationFunctionType
    ALU = mybir.AluOpType
    AX = mybir.AxisListType

    nc = bacc.Bacc(
        "TRN2", target_bir_lowering=False, debug=False, enable_asserts=False
    )

    x_d = nc.dram_tensor("x", [C, N], f32, kind="ExternalInput").ap()
    wqkv_d = nc.dram_tensor("wqkvT", [C, 3 * C], f32, kind="ExternalInput").ap()
    wproj_d = nc.dram_tensor("wprojT", [C, C], f32, kind="ExternalInput").ap()
    wn_d = nc.dram_tensor("wn", [CT, P], f32, kind="ExternalInput").ap()
    bn_d = nc.dram_tensor("bn", [CT, P], f32, kind="ExternalInput").ap()
    pb_d = nc.dram_tensor("pb", [CT, P], f32, kind="ExternalInput").ap()
    vb_d = nc.dram_tensor("vb", [P, C], f32, kind="ExternalInput").ap()
    pmat_d = nc.dram_tensor("pmat", [P, P], f32, kind="ExternalInput").ap()
    ones_d = nc.dram_tensor("ones", [P, 1], f32, kind="ExternalInput").ap()
    y_d = nc.dram_tensor("y", [C, N], f32, kind="ExternalOutput").ap()

    def r(ap):
        return ap.bitcast(f32r)

    with tile.TileContext(nc) as tc:
        with (
            tc.tile_pool(name="consts", bufs=1) as consts,
            tc.tile_pool(name="persist", bufs=1) as persist,
            ExitStack() as late_pools,
        ):
            # --- constants into SBUF ---
            wqkv_s = consts.tile([P, CT, 3 * C], f32, name="wqkv_s")
            nc.sync.dma_start(
                out=r(wqkv_s), in_=r(wqkv_d.rearrange("(t p) o -> p t o", p=P))
            )
            wproj_s = consts.tile([P, CT, C], f32, name="wproj_s")
            nc.sync.dma_start(
                out=r(wproj_s), in_=r(wproj_d.rearrange("(t p) o -> p t o", p=P))
            )
            pmat_s = consts.tile([P, P], f32, name="pmat_s")
            nc.sync.dma_start(out=pmat_s, in_=pmat_d)
            vb_s = consts.tile([P, C], f32, name="vb_s")
            nc.sync.dma_start(out=vb_s, in_=vb_d)
            wn_s = consts.tile([P, CT], f32, name="wn_s")
            nc.sync.dma_start(out=wn_s, in_=wn_d.rearrange("t p -> p t"))
            bn_s = consts.tile([P, CT], f32, name="bn_s")
            nc.sync.dma_start(out=bn_s, in_=bn_d.rearrange("t p -> p t"))
            pb_s = consts.tile([P, CT], f32, name="pb_s")
            nc.sync.dma_start(out=pb_s, in_=pb_d.rearrange("t p -> p t"))
            eps_s = consts.tile([P, 1], f32, name="eps_s")
            nc.vector.memset(eps_s, EPS)
            ones_s = consts.tile([P, 1], f32, name="ones_s")
            nc.vector.memset(ones_s, 1.0)

            # --- long-lived tensors ---
            h_s = persist.tile([P, CT, N], f32, name="h_s")  # 64KB/p
            ctx1_s = persist.tile([P, CT, C], f32, name="ctx1_s")
            mts_s = persist.tile([P, CT, C], f32, name="mts_s")
            A_s = persist.tile([P, CT], f32, name="A_s")
            B_s = persist.tile([P, CT], f32, name="B_s")
            rk_s = persist.tile([P, CT], f32, name="rk_s")
            sumq_parts = persist.tile([P, CT, NCH], f32, name="sumq_parts")
            sumq_s = persist.tile([P, CT], f32, name="sumq_s")
            rq_s = persist.tile([P, CT], f32, name="rq_s")

            # ---------- Phase 1: GroupNorm stats + apply ----------
            with (
                tc.tile_pool(name="xin", bufs=2) as xin,
                tc.tile_pool(name="gn_sm", bufs=8) as gnsm,
                tc.tile_pool(name="gn_ps", bufs=2, space="PSUM") as gnps,
            ):
                for j in range(CT):
                    x_t = xin.tile([P, N], f32, name="x_t")
                    nc.sync.dma_start(out=x_t, in_=x_d[ts(j, P), :])
                    stats = gnsm.tile([P, 2], f32, name="stats")
                    # per-partition sum (DVE) and sum-of-squares (ACT, main
                    # out is scratch into h which gets overwritten below)
                    nc.vector.tensor_reduce(
                        out=stats[:, 0:1], in_=x_t, axis=AX.X, op=ALU.add
                    )
                    nc.scalar.activation(
                        out=r(h_s[:, j, :]),
                        in_=x_t,
                        func=AF.Square,
                        accum_out=stats[:, 1:2],
                    )
                    # group-sum + broadcast back to member partitions in one
                    # matmul with the block-diagonal indicator matrix
                    gps = gnps.tile([P, 2], f32, name="gps")
                    nc.tensor.matmul(gps, lhsT=pmat_s, rhs=stats, start=True, stop=True)
                    mv = gnsm.tile([P, 2], f32, name="mv")
                    nc.vector.tensor_scalar_mul(
                        out=mv, in0=gps, scalar1=1.0 / (GSIZE * N)
                    )
                    musq = gnsm.tile([P, 1], f32, name="musq")
                    nc.vector.tensor_mul(out=musq, in0=mv[:, 0:1], in1=mv[:, 0:1])
                    var = gnsm.tile([P, 1], f32, name="var")
                    nc.vector.tensor_sub(out=var, in0=mv[:, 1:2], in1=musq)
                    std = gnsm.tile([P, 1], f32, name="std")
                    nc.scalar.activation(
                        out=std, in_=var, func=AF.Sqrt, bias=eps_s, scale=1.0
                    )
                    rstd = gnsm.tile([P, 1], f32, name="rstd")
                    nc.vector.reciprocal(out=rstd, in_=std)
                    # A = rstd*w ; B = b - mu*A
                    nc.vector.tensor_mul(
                        out=A_s[:, j : j + 1], in0=rstd, in1=wn_s[:, j : j + 1]
                    )
                    muA = gnsm.tile([P, 1], f32, name="muA")
                    nc.vector.tensor_mul(
                        out=muA, in0=mv[:, 0:1], in1=A_s[:, j : j + 1]
                    )
                    nc.vector.tensor_sub(
                        out=B_s[:, j : j + 1], in0=bn_s[:, j : j + 1], in1=muA
                    )
                    # h = x*A + B
                    nc.scalar.activation(
                        out=r(h_s[:, j, :]),
                        in_=x_t,
                        func=AF.Identity,
                        bias=B_s[:, j : j + 1],
                        scale=A_s[:, j : j + 1],
                    )

            # expq allocated only now: the stack allocator reuses the SBUF
            # freed by the phase-1 x pool (which closed above)
            eqp = late_pools.enter_context(tc.tile_pool(name="eq", bufs=1))
            expq_s = eqp.tile([P, CT, N], f32, name="expq_s")  # 64KB/p

            # ---------- Phase 2a: k/v (transposed) + context accumulation ----------
            with tc.tile_pool(name="ctxps", bufs=1, space="PSUM") as ctxps:
                ctx_ps = [
                    ctxps.tile([P, C], f32, name=f"ctx_ps{j}") for j in range(CT)
                ]
                sumk_ps = ctxps.tile([P, CT], f32, name="sumk_ps")
                with (
                    tc.tile_pool(name="kvps", bufs=2, space="PSUM") as kvps,
                    tc.tile_pool(name="kvsb", bufs=3) as kvsb,
                ):
                    for i in range(NT):
                        kt_ps = kvps.tile([P, C], f32, name="kt_ps", tag="kv")
                        for j in range(CT):
                            nc.tensor.matmul(
                                kt_ps,
                                lhsT=r(h_s[:, j, ts(i, P)]),
                                rhs=r(wqkv_s[:, j, C : 2 * C]),
                                start=(j == 0),
                                stop=(j == CT - 1),
                            )
                        ekt = kvsb.tile([P, C], f32, name="ekt")
                        nc.scalar.activation(out=r(ekt), in_=kt_ps, func=AF.Exp)
                        vt_ps = kvps.tile([P, C], f32, name="vt_ps", tag="kv")
                        for j in range(CT):
                            nc.tensor.matmul(
                                vt_ps,
                                lhsT=r(h_s[:, j, ts(i, P)]),
                                rhs=r(wqkv_s[:, j, 2 * C : 3 * C]),
                                start=(j == 0),
                                stop=(j == CT - 1),
                            )
                        vt = kvsb.tile([P, C], f32, name="vt")
                        nc.scalar.copy(out=r(vt), in_=vt_ps)
                        for j in range(CT):
                            nc.tensor.matmul(
                                ctx_ps[j],
                                lhsT=r(ekt[:, ts(j, P)]),
                                rhs=r(vt),
                                start=(i == 0),
                                stop=(i == NT - 1),
                            )
                            nc.tensor.matmul(
                                sumk_ps[:, j : j + 1],
                                lhsT=ekt[:, ts(j, P)],
                                rhs=ones_s,
                                start=(i == 0),
                                stop=(i == NT - 1),
                            )

                # normalize ctx rows by 1/sumk, add v-bias rank-1 correction
                nc.vector.reciprocal(out=rk_s, in_=sumk_ps)
                for j in range(CT):
                    nc.vector.scalar_tensor_tensor(
                        out=r(ctx1_s[:, j, :]),
                        in0=ctx_ps[j],
                        scalar=rk_s[:, j : j + 1],
                        in1=vb_s,
                        op0=ALU.mult,
                        op1=ALU.add,
                    )

                # ---------- Phase 2b: q (natural layout) + exp + row sums ----------
                with tc.tile_pool(name="qps", bufs=2, space="PSUM") as qps:
                    for t in range(CT):
                        for m in range(NCH):
                            q_ps = qps.tile([P, 512], f32, name="q_ps")
                            for j in range(CT):
                                nc.tensor.matmul(
                                    q_ps,
                                    lhsT=r(wqkv_s[:, j, ts(t, P)]),
                                    rhs=r(h_s[:, j, ts(m, 512)]),
                                    start=(j == 0),
                                    stop=(j == CT - 1),
                                )
                            nc.scalar.activation(
                                out=r(expq_s[:, t, ts(m, 512)]),
                                in_=q_ps,
                                func=AF.Exp,
                                accum_out=sumq_parts[:, t, m : m + 1],
                            )
                    nc.vector.tensor_reduce(
                        out=sumq_s, in_=sumq_parts, axis=AX.X, op=ALU.add
                    )
                    nc.vector.reciprocal(out=rq_s, in_=sumq_s)
                    nc.vector.tensor_scalar_mul(
                        out=rq_s, in0=rq_s, scalar1=float(C) ** -0.5
                    )

            # ---------- Phase 3: MT = (proj_w @ ctx')^T with row scales ----------
            with tc.tile_pool(name="mtps", bufs=2, space="PSUM") as mtps:
                for dt in range(CT):
                    mt_ps = mtps.tile([P, C], f32, name="mt_ps")
                    for j in range(CT):
                        nc.tensor.matmul(
                            mt_ps,
                            lhsT=r(ctx1_s[:, j, ts(dt, P)]),
                            rhs=r(wproj_s[:, j, :]),
                            start=(j == 0),
                            stop=(j == CT - 1),
                        )
                    nc.vector.tensor_scalar_mul(
                        out=r(mts_s[:, dt, :]), in0=mt_ps, scalar1=rq_s[:, dt : dt + 1]
                    )

            # ---------- Phase 4: final GEMM + proj bias + residual ----------
            with (
                tc.tile_pool(name="fps", bufs=2, space="PSUM") as fps,
                tc.tile_pool(name="xr", bufs=3) as xrp,
                tc.tile_pool(name="outp", bufs=3) as outp,
            ):
                for t in range(CT):
                    for m in range(NCH):
                        f_ps = fps.tile([P, 512], f32, name="f_ps")
                        for dt in range(CT):
                            nc.tensor.matmul(
                                f_ps,
                                lhsT=r(mts_s[:, dt, ts(t, P)]),
                                rhs=r(expq_s[:, dt, ts(m, 512)]),
                                start=(dt == 0),
                                stop=(dt == CT - 1),
                            )
                        xr = xrp.tile([P, 512], f32, name="xr")
                        nc.sync.dma_start(
                            out=xr, in_=x_d[ts(t, P), ts(m, 512)]
                        )
                        ot = outp.tile([P, 512], f32, name="ot")
                        nc.vector.scalar_tensor_tensor(
                            out=ot,
                            in0=f_ps,
                            scalar=pb_s[:, t : t + 1],
                            in1=xr,
                            op0=ALU.add,
                            op1=ALU.add,
                        )
                        nc.sync.dma_start(
                            out=y_d[ts(t, P), ts(m, 512)], in_=ot
                        )

    nc.compile()
    return nc


def kernel(x, norm_w, norm_b, qkv_w, qkv_b, proj_w, proj_b):
    from concourse.bass_utils import run_bass_kernel_spmd

    x = np.ascontiguousarray(np.asarray(x, dtype=np.float32))
    norm_w = np.asarray(norm_w, dtype=np.float32)
    norm_b = np.asarray(norm_b, dtype=np.float32)
    qkv_w = np.asarray(qkv_w, dtype=np.float32)
    qkv_b = np.asarray(qkv_b, dtype=np.float32)
    proj_w = np.asarray(proj_w, dtype=np.float32)
    proj_b = np.asarray(proj_b, dtype=np.float32)

    if "nc" not in _CACHE:
        _CACHE["nc"] = _build_program()
    nc = _CACHE["nc"]

    xf = x.reshape(B, C, N)
    wqkvT = np.ascontiguousarray(qkv_w.T)  # [C, 3C]
    wprojT = np.ascontiguousarray(proj_w.T)  # [C, C]
    wn = np.ascontiguousarray(norm_w.reshape(CT, P))
    bn = np.ascontiguousarray(norm_b.reshape(CT, P))
    pb = np.ascontiguousarray(proj_b.reshape(CT, P))
    vb = np.ascontiguousarray(
        np.broadcast_to(qkv_b[2 * C : 3 * C], (P, C))
    ).astype(np.float32)
    pmat = np.kron(
        np.eye(P // GSIZE, dtype=np.float32), np.ones((GSIZE, GSIZE), np.float32)
    )

    shared = {
        "wqkvT": wqkvT,
        "wprojT": wprojT,
        "wn": wn,
        "bn": bn,
        "pb": pb,
        "vb": vb,
        "pmat": pmat,
        "ones": np.ones((P, 1), np.float32),
    }
    in_maps = [dict(shared, x=np.ascontiguousarray(xf[b])) for b in range(B)]

    trace = bool(int(os.environ.get("BASS_ATTN_PROFILE", "0")))
    res = run_bass_kernel_spmd(
        nc, in_maps, core_ids=list(range(B)), trace=trace
    )
    if trace and res.exec_time_ns is not None:
        print(f"HW exec time: {res.exec_time_ns} ns")

    out = np.stack([res.results[b]["y"] for b in range(B)], axis=0)
    return out.reshape(B, C, H, W)


# revision 15
# speedup vs baseline: 1.1328x; 1.1328x over previous
"""AttentionBlock (GroupNorm + linear attention + proj + residual) on 8 Trainium2 cores.

Reference computation (per batch element b, C=512, HW=4096):
    h   = GroupNorm32(x) * w + b
    qkv = qkv_w @ h                       (1x1 conv == channel matmul)
    q   = softmax(q, axis=spatial) * C^-0.5
    k   = softmax(k, axis=spatial)
    ctx = k @ v^T                         [C, C]
    out = proj_w @ (ctx @ q) + proj_b + x

Sharding: data-parallel over batch B=8 -> one batch element per NeuronCore.

Kernel algebra (per core):
  - softmax(q+qb) == softmax(q): per-row bias shifts cancel; only v's qkv-bias
    matters and enters as a rank-1 correction to ctx (ctx += vb[d]).
  - exp() without max-subtraction (q,k values are O(1)); softmax denominators
    (sumq, sumk) folded into row scales of small [C,C] matrices.
  - proj_w folded in early: MT = (proj_w @ ctx')^T, so the last big GEMM is
    MT @ expq and the separate proj GEMM disappears.
  - k and v are produced directly in [n, c] (transposed) layout by using the
    h-tile as the matmul's stationary operand; no explicit transposes anywhere.
  - all large matmuls run as float32r (full PE rate at N=512, ~fp32 accuracy).
"""

import os
from contextlib import ExitStack

import numpy as np

B = 8
C = 512
H = W = 64
N = H * W  # 4096 spatial positions
P = 128  # partitions
CT = C // P  # 4 channel tiles
NT = N // P  # 32 spatial tiles of 128 (for transposed k/v)
NCH = N // 512  # 8 spatial chunks of 512
GROUPS = 32
GSIZE = C // GROUPS  # 16 channels per group
EPS = 1e-5

_CACHE = {}


def _build_program():
    import concourse.bass as bass
    import concourse.tile as tile
    from concourse import bacc, mybir
    from concourse.bass import ts

    f32 = mybir.dt.float32
    f32r = mybir.dt.float32r
    AF = mybir.ActivationFunctionType
    ALU = mybir.AluOpType
    AX = mybir.AxisListType

    nc = bacc.Bacc(
        "TRN2", target_bir_lowering=False, debug=False, enable_asserts=False
    )

    x_d = nc.dram_tensor("x", [C, N], f32, kind="ExternalInput").ap()
    wqkv_d = nc.dram_tensor("wqkvT", [C, 3 * C], f32, kind="ExternalInput").ap()
    wproj_d = nc.dram_tensor("wprojT", [C, C], f32, kind="ExternalInput").ap()
    wn_d = nc.dram_tensor("wn", [CT, P], f32, kind="ExternalInput").ap()
    bn_d = nc.dram_tensor("bn", [CT, P], f32, kind="ExternalInput").ap()
    pb_d = nc.dram_tensor("pb", [CT, P], f32, kind="ExternalInput").ap()
    vb_d = nc.dram_tensor("vb", [P, C], f32, kind="ExternalInput").ap()
    pmat_d = nc.dram_tensor("pmat", [P, P], f32, kind="ExternalInput").ap()
    ones_d = nc.dram_tensor("ones", [P, 1], f32, kind="ExternalInput").ap()
    y_d = nc.dram_tensor("y", [C, N], f32, kind="ExternalOutput").ap()

    def r(ap):
        return ap.bitcast(f32r)

    with tile.TileContext(nc) as tc:
        with (
            tc.tile_pool(name="consts", bufs=1) as consts,
            tc.tile_pool(name="persist", bufs=1) as persist,
            ExitStack() as late_pools,
        ):
            # --- constants into SBUF ---
            wqkv_s = consts.tile([P, CT, 3 * C], f32, name="wqkv_s")
            nc.sync.dma_start(
                out=r(wqkv_s), in_=r(wqkv_d.rearrange("(t p) o -> p t o", p=P))
            )
            wproj_s = consts.tile([P, CT, C], f32, name="wproj_s")
            nc.sync.dma_start(
                out=r(wproj_s), in_=r(wproj_d.rearrange("(t p) o -> p t o", p=P))
            )
            pmat_s = consts.tile([P, P], f32, name="pmat_s")
            nc.sync.dma_start(out=pmat_s, in_=pmat_d)
            vb_s = consts.tile([P, C], f32, name="vb_s")
            nc.sync.dma_start(out=vb_s, in_=vb_d)
            wn_s = consts.tile([P, CT], f32, name="wn_s")
            nc.sync.dma_start(out=wn_s, in_=wn_d.rearrange("t p -> p t"))
            bn_s = consts.tile([P, CT], f32, name="bn_s")
            nc.sync.dma_start(out=bn_s, in_=bn_d.rearrange("t p -> p t"))
            pb_s = consts.tile([P, CT], f32, name="pb_s")
            nc.sync.dma_start(out=pb_s, in_=pb_d.rearrange("t p -> p t"))
            eps_s = consts.tile([P, 1], f32, name="eps_s")
            nc.vector.memset(eps_s, EPS)
            ones_s = consts.tile([P, 1], f32, name="ones_s")
            nc.vector.memset(ones_s, 1.0)

            # --- long-lived tensors ---
            h_s = persist.tile([P, CT, N], f32, name="h_s")  # 64KB/p
            ctx1_s = persist.tile([P, CT, C], f32, name="ctx1_s")
            mts_s = persist.tile([P, CT, C], f32, name="mts_s")
            A_s = persist.tile([P, CT], f32, name="A_s")
            B_s = persist.tile([P, CT], f32, name="B_s")
            rk_s = persist.tile([P, CT], f32, name="rk_s")
            sumq_parts = persist.tile([P, CT, NCH], f32, name="sumq_parts")
            sumq_s = persist.tile([P, CT], f32, name="sumq_s")
            rq_s = persist.tile([P, CT], f32, name="rq_s")

            # ---------- Phase 1: GroupNorm stats + apply ----------
            with (
                tc.tile_pool(name="xin", bufs=2) as xin,
                tc.tile_pool(name="gn_sm", bufs=8) as gnsm,
                tc.tile_pool(name="gn_ps", bufs=2, space="PSUM") as gnps,
            ):
                for j in range(CT):
                    x_t = xin.tile([P, N], f32, name="x_t")
                    nc.sync.dma_start(out=x_t, in_=x_d[ts(j, P), :])
                    stats = gnsm.tile([P, 2], f32, name="stats")
                    # per-partition sum (DVE) and sum-of-squares (ACT, main
                    # out is scratch into h which gets overwritten below)
                    nc.vector.tensor_reduce(
                        out=stats[:, 0:1], in_=x_t, axis=AX.X, op=ALU.add
                    )
                    nc.scalar.activation(
                        out=r(h_s[:, j, :]),
                        in_=x_t,
                        func=AF.Square,
                        accum_out=stats[:, 1:2],
                    )
                    # group-sum + broadcast back to member partitions in one
                    # matmul with the block-diagonal indicator matrix
                    gps = gnps.tile([P, 2], f32, name="gps")
                    nc.tensor.matmul(gps, lhsT=pmat_s, rhs=stats, start=True, stop=True)
                    mv = gnsm.tile([P, 2], f32, name="mv")
                    nc.vector.tensor_scalar_mul(
                        out=mv, in0=gps, scalar1=1.0 / (GSIZE * N)
                    )
                    musq = gnsm.tile([P, 1], f32, name="musq")
                    nc.vector.tensor_mul(out=musq, in0=mv[:, 0:1], in1=mv[:, 0:1])
                    var = gnsm.tile([P, 1], f32, name="var")
                    nc.vector.tensor_sub(out=var, in0=mv[:, 1:2], in1=musq)
                    std = gnsm.tile([P, 1], f32, name="std")
                    nc.scalar.activation(
                        out=std, in_=var, func=AF.Sqrt, bias=eps_s, scale=1.0
                    )
                    rstd = gnsm.tile([P, 1], f32, name="rstd")
                    nc.vector.reciprocal(out=rstd, in_=std)
                    # A = rstd*w ; B = b - mu*A
                    nc.vector.tensor_mul(
                        out=A_s[:, j : j + 1], in0=rstd, in1=wn_s[:, j : j + 1]
                    )
                    muA = gnsm.tile([P, 1], f32, name="muA")
                    nc.vector.tensor_mul(
                        out=muA, in0=mv[:, 0:1], in1=A_s[:, j : j + 1]
                    )
                    nc.vector.tensor_sub(
                        out=B_s[:, j : j + 1], in0=bn_s[:, j : j + 1], in1=muA
                    )
                    # h = x*A + B
                    nc.scalar.activation(
                        out=r(h_s[:, j, :]),
                        in_=x_t,
                        func=AF.Identity,
                        bias=B_s[:, j : j + 1],
                        scale=A_s[:, j : j + 1],
                    )

            # expq allocated only now: the stack allocator reuses the SBUF
            # freed by the phase-1 x pool (which closed above)
            eqp = late_pools.enter_context(tc.tile_pool(name="eq", bufs=1))
            expq_s = eqp.tile([P, CT, N], f32, name="expq_s")  # 64KB/p

            # ---------- Phase 2a: k/v (transposed) + context accumulation ----------
            with tc.tile_pool(name="ctxps", bufs=1, space="PSUM") as ctxps:
                ctx_ps = [
                    ctxps.tile([P, C], f32, name=f"ctx_ps{j}") for j in range(CT)
                ]
                sumk_ps = ctxps.tile([P, CT], f32, name="sumk_ps")
                with (
                    tc.tile_pool(name="kvps", bufs=2, space="PSUM") as kvps,
                    tc.tile_pool(name="kvsb", bufs=3) as kvsb,
                ):
                    for i in range(NT):
                        kt_ps = kvps.tile([P, C], f32, name="kt_ps", tag="kv")
                        for j in range(CT):
                            nc.tensor.matmul(
                                kt_ps,
                                lhsT=r(h_s[:, j, ts(i, P)]),
                                rhs=r(wqkv_s[:, j, C : 2 * C]),
                                start=(j == 0),
                                stop=(j == CT - 1),
                            )
                        ekt = kvsb.tile([P, C], f32, name="ekt")
                        nc.scalar.activation(out=r(ekt), in_=kt_ps, func=AF.Exp)
                        vt_ps = kvps.tile([P, C], f32, name="vt_ps", tag="kv")
                        for j in range(CT):
                            nc.tensor.matmul(
                                vt_ps,
                                lhsT=r(h_s[:, j, ts(i, P)]),
                                rhs=r(wqkv_s[:, j, 2 * C : 3 * C]),
                                start=(j == 0),
                                stop=(j == CT - 1),
                            )
                        vt = kvsb.tile([P, C], f32, name="vt")
                        nc.scalar.copy(out=r(vt), in_=vt_ps)
                        for j in range(CT):
                            nc.tensor.matmul(
                                ctx_ps[j],
                                lhsT=r(ekt[:, ts(j, P)]),
                                rhs=r(vt),
                                start=(i == 0),
                                stop=(i == NT - 1),
                            )
                            nc.tensor.matmul(
                                sumk_ps[:, j : j + 1],
                                lhsT=ekt[:, ts(j, P)],
                                rhs=ones_s,
                                start=(i == 0),
                                stop=(i == NT - 1),
                            )

                # normalize ctx rows by 1/sumk, add v-bias rank-1 correction
                nc.vector.reciprocal(out=rk_s, in_=sumk_ps)
                for j in range(CT):
                    nc.vector.scalar_tensor_tensor(
                        out=r(ctx1_s[:, j, :]),
                        in0=ctx_ps[j],
                        scalar=rk_s[:, j : j + 1],
                        in1=vb_s,
                        op0=ALU.mult,
                        op1=ALU.add,
                    )

                # ---------- Phase 2b: q (natural layout) + exp + row sums ----------
                with tc.tile_pool(name="qps", bufs=2, space="PSUM") as qps:
                    for t in range(CT):
                        for m in range(NCH):
                            q_ps = qps.tile([P, 512], f32, name="q_ps")
                            for j in range(CT):
                                nc.tensor.matmul(
                                    q_ps,
                                    lhsT=r(wqkv_s[:, j, ts(t, P)]),
                                    rhs=r(h_s[:, j, ts(m, 512)]),
                                    start=(j == 0),
                                    stop=(j == CT - 1),
                                )
                            nc.scalar.activation(
                                out=r(expq_s[:, t, ts(m, 512)]),
                                in_=q_ps,
                                func=AF.Exp,
                                accum_out=sumq_parts[:, t, m : m + 1],
                            )
                    nc.vector.tensor_reduce(
                        out=sumq_s, in_=sumq_parts, axis=AX.X, op=ALU.add
                    )
                    nc.vector.reciprocal(out=rq_s, in_=sumq_s)
                    nc.vector.tensor_scalar_mul(
                        out=rq_s, in0=rq_s, scalar1=float(C) ** -0.5
                    )

            # ---------- Phase 3: MT = (proj_w @ ctx')^T with row scales ----------
            with tc.tile_pool(name="mtps", bufs=2, space="PSUM") as mtps:
                for dt in range(CT):
                    mt_ps = mtps.tile([P, C], f32, name="mt_ps")
                    for j in range(CT):
                        nc.tensor.matmul(
                            mt_ps,
                            lhsT=r(ctx1_s[:, j, ts(dt, P)]),
                            rhs=r(wproj_s[:, j, :]),
                            start=(j == 0),
                            stop=(j == CT - 1),
                        )
                    nc.vector.tensor_scalar_mul(
                        out=r(mts_s[:, dt, :]), in0=mt_ps, scalar1=rq_s[:, dt : dt + 1]
                    )

            # ---------- Phase 4: final GEMM + proj bias + residual ----------
            with (
                tc.tile_pool(name="fps", bufs=2, space="PSUM") as fps,
                tc.tile_pool(name="xr", bufs=3) as xrp,
                tc.tile_pool(name="outp", bufs=3) as outp,
            ):
                for t in range(CT):
                    for m in range(NCH):
                        f_ps = fps.tile([P, 512], f32, name="f_ps")
                        for dt in range(CT):
                            nc.tensor.matmul(
                                f_ps,
                                lhsT=r(mts_s[:, dt, ts(t, P)]),
                                rhs=r(expq_s[:, dt, ts(m, 512)]),
                                start=(dt == 0),
                                stop=(dt == CT - 1),
                            )
                        xr = xrp.tile([P, 512], f32, name="xr")
                        nc.sync.dma_start(
                            out=xr, in_=x_d[ts(t, P), ts(m, 512)]
                        )
                        ot = outp.tile([P, 512], f32, name="ot")
                        nc.vector.scalar_tensor_tensor(
                            out=ot,
                            in0=f_ps,
                            scalar=pb_s[:, t : t + 1],
                            in1=xr,
                            op0=ALU.add,
                            op1=ALU.add,
                        )
                        nc.sync.dma_start(
                            out=y_d[ts(t, P), ts(m, 512)], in_=ot
                        )

    nc.compile()
    return nc


def kernel(x, norm_w, norm_b, qkv_w, qkv_b, proj_w, proj_b):
    from concourse.bass_utils import run_bass_kernel_spmd

    x = np.ascontiguousarray(np.asarray(x, dtype=np.float32))
    norm_w = np.asarray(norm_w, dtype=np.float32)
    norm_b = np.asarray(norm_b, dtype=np.float32)
    qkv_w = np.asarray(qkv_w, dtype=np.float32)
    qkv_b = np.asarray(qkv_b, dtype=np.float32)
    proj_w = np.asarray(proj_w, dtype=np.float32)
    proj_b = np.asarray(proj_b, dtype=np.float32)

    if "nc" not in _CACHE:
        _CACHE["nc"] = _build_program()
    nc = _CACHE["nc"]

    xf = x.reshape(B, C, N)
    wqkvT = np.ascontiguousarray(qkv_w.T)  # [C, 3C]
    wprojT = np.ascontiguousarray(proj_w.T)  # [C, C]
    wn = np.ascontiguousarray(norm_w.reshape(CT, P))
    bn = np.ascontiguousarray(norm_b.reshape(CT, P))
    pb = np.ascontiguousarray(proj_b.reshape(CT, P))
    vb = np.ascontiguousarray(
        np.broadcast_to(qkv_b[2 * C : 3 * C], (P, C))
    ).astype(np.float32)
    pmat = np.kron(
        np.eye(P // GSIZE, dtype=np.float32), np.ones((GSIZE, GSIZE), np.float32)
    )

    shared = {
        "wqkvT": wqkvT,
        "wprojT": wprojT,
        "wn": wn,
        "bn": bn,
        "pb": pb,
        "vb": vb,
        "pmat": pmat,
        "ones": np.ones((P, 1), np.float32),
    }
    in_maps = [dict(shared, x=np.ascontiguousarray(xf[b])) for b in range(B)]

    trace = bool(int(os.environ.get("BASS_ATTN_PROFILE", "0")))
    res = run_bass_kernel_spmd(
        nc, in_maps, core_ids=list(range(B)), trace=trace
    )
    _CACHE["last_result"] = res
    if trace and res.exec_time_ns is not None:
        print(f"HW exec time: {res.exec_time_ns} ns")

    out = np.stack([res.results[b]["y"] for b in range(B)], axis=0)
    return out.reshape(B, C, H, W)
